# revision 11
# baseline (speedup 1.0000x reference)
"""Paged-KV-cache causal GQA attention on 8 TRN2 NeuronCores.

Problem shape (hardcoded): B=8 seqs x S=1024 tokens, H=32 q-heads,
KVH=8 kv-heads (GQA group 4), D=128, block_size=256, 40 cache blocks.

Sharding: data parallel, one sequence per core. Host does the
store_kvcache scatter + block-table gather (layout work) and per-core
layout prep (head-major transposes + bf16 cast, scale folded into q);
each core runs causal flash attention for its sequence over all 32
heads.

Device algorithm per (head, q-chunk of 512), two heads interleaved:
  warmup: 8 dummy matmuls on a memset tile bridge the initial DMA
          wait so the PE HAM clock-gate reaches 8/8 before real work
  phase 1 (per k-tile group of 2-4 tiles): scores^T[k,q] = K^T.T @ Q^T
           (PE, bf16) packed into one [128, <=1536] psum tile (regions
           never cross a psum bank boundary); P = exp(scores) in ONE
           wide ACT inst per group (ACT is the global bottleneck:
           fewer/wider exps cut its ~176ns/inst overhead); one group
           per pair instead runs a bf16-Schraudolph exp on the DVE to
           offload ACT; diagonal tiles masked into separate tiles (DVE)
  phase 2 (per q-tile): O[q,0:128]+rowsum[q] = P.T @ [V|1] accumulated
           over its k tiles back-to-back (PE), then out = O * (1/rowsum)
           (one DVE op per q-tile pair via broadcast AP, bf16 out) and
           DMA out (bf16 halves store traffic; host upcasts to f32).
Score psum double-buffered 2x3 banks + po double-buffered 2x1 bank.
The head phase is HBM-bound (8 cores burst-load at once): critical
bytes go on one ring in strict need order; v1[0] rides the scalar ring.
"""

import sys

import numpy as np
import ml_dtypes

sys.path.insert(0, "/opt/trn_rl_repo")

import concourse.bass as bass  # noqa: E402
import concourse.mybir as mybir  # noqa: E402
import concourse.tile as tile  # noqa: E402
from concourse import bacc  # noqa: E402
from concourse.bass_utils import run_bass_kernel_spmd  # noqa: E402

B, S = 8, 1024
H, KVH, D = 32, 8, 128
G = H // KVH
NT = S // 128  # 8 k/q tiles of 128 per sequence
VW = 132  # v tile row: 128 v cols + ones col + pad
SCALE = 1.0 / float(np.sqrt(D))
BF = mybir.dt.bfloat16
F32 = mybir.dt.float32
_NC = None

# k-tile groups per q-chunk: (qc, [(kt, q_off, width, psum_off), ...], tw).
# psum_off values are arranged so no matmul output region crosses a 2KB
# (512-f32) psum bank boundary.
GROUPS = [
    (0, [(0, 0, 512, 0), (1, 1, 384, 512), (3, 3, 128, 896), (2, 2, 256, 1024)], 1280),
    (1, [(0, 0, 512, 0), (1, 0, 512, 512), (2, 0, 512, 1024)], 1536),
    (1, [(3, 0, 512, 0), (4, 0, 512, 512)], 1024),
    (1, [(5, 1, 384, 0), (7, 3, 128, 384), (6, 2, 256, 512)], 768),
]
# last unit: split kt5 out so qt5's PV fully finalizes before qt6/7's
# pre-accumulation allocates its psum slot (pool-rotation safety), and
# only the kt6/kt7 diagonal matmuls trail the final exp.
GROUPS_LAST = [
    GROUPS[0],
    GROUPS[1],
    GROUPS[2],
    (1, [(5, 1, 384, 0)], 384),
    (1, [(6, 2, 256, 0), (7, 3, 128, 256)], 384),
]


def _build_nc():
    nc = bacc.Bacc("TRN2", target_bir_lowering=False, debug=False, num_devices=8)
    qT = nc.dram_tensor("qT", [H, D, S], BF, kind="ExternalInput").ap()
    kT = nc.dram_tensor("kT", [KVH, D, S], BF, kind="ExternalInput").ap()
    v1 = nc.dram_tensor("v1", [KVH, NT, 128, VW], BF, kind="ExternalInput").ap()
    out = nc.dram_tensor("out", [H, S, D], BF, kind="ExternalOutput").ap()
    mask_np = np.triu(np.ones((128, 128), dtype=ml_dtypes.bfloat16))
    mask_dram = nc.inline_tensor(mask_np, "tri_mask").ap()

    with tile.TileContext(nc) as tc:
        with (
            tc.tile_pool(name="singles", bufs=1) as singles,
            tc.tile_pool(name="qpool", bufs=6) as qpool,
            tc.tile_pool(name="ppool", bufs=16) as ppool,
            tc.tile_pool(name="dpool", bufs=22) as dpool,
            tc.tile_pool(name="opool", bufs=8) as opool,
            tc.tile_pool(name="rpool", bufs=10) as rpool,
            tc.tile_pool(name="pspool", bufs=2, space="PSUM") as pspool,
            tc.tile_pool(name="popool", bufs=2, space="PSUM") as popool,
        ):
            # --- HAM warmup: dummy matmuls with no data deps keep the
            # PE busy through the initial DMA wait so the clock gate is
            # at 8/8 when the first real matmul issues ---
            warm_sb = singles.tile([128, 256], BF, name="warm_sb")
            nc.vector.memset(warm_sb, 0.0)
            dummy_ps = popool.tile([128, 258], F32, tag="po", name="dummy_ps")
            for i in range(20):
                nc.tensor.matmul(
                    dummy_ps[:, 0:256],
                    lhsT=warm_sb[:, 0:128],
                    rhs=warm_sb,
                    start=True,
                    stop=True,
                    skip_group_check=True,
                )

            mask_sb = singles.tile([128, 128], BF)
            kv_sb = []
            for kvh in range(KVH):
                k_t = singles.tile([128, S], BF, name=f"kT_sb{kvh}", tag=f"kT{kvh}")
                v_t = singles.tile(
                    [128, NT * VW], BF, name=f"v1_sb{kvh}", tag=f"v1{kvh}"
                )
                kv_sb.append((k_t, v_t))

            def load_kv(kvh):
                # kT on the sync HWDGE ring; v1 on the gpsimd SWDGE ring so
                # the two streams' kickoffs and transfers run in parallel
                nc.sync.dma_start(out=kv_sb[kvh][0], in_=kT[kvh])
                nc.gpsimd.dma_start(
                    out=kv_sb[kvh][1].rearrange("p (t c) -> p t c", t=NT),
                    in_=v1[kvh].rearrange("t p c -> p t c"),
                )

            q_tiles = {}

            def load_q(h):
                if h < H and h not in q_tiles:
                    q_tiles[h] = qpool.tile([128, S], BF, tag="q", name=f"q_sb{h}")
                    nc.sync.dma_start(out=q_tiles[h], in_=qT[h])

            # fast start: the head phase is HBM-bandwidth-bound (all 8 cores
            # burst-load at once), so the critical bytes go on ONE ring in
            # strict need order and nothing else competes with them; only
            # v1[0] (needed ~1us later, for the first PVs) rides the
            # otherwise-idle scalar HWDGE ring
            q_tiles[0] = qpool.tile([128, S], BF, tag="q", name="q_sb0")
            q_tiles[1] = qpool.tile([128, S], BF, tag="q", name="q_sb1")
            nc.sync.dma_start(out=kv_sb[0][0][:, 0:512], in_=kT[0][:, 0:512])
            nc.sync.dma_start(out=q_tiles[0][:, 0:512], in_=qT[0][:, 0:512])
            nc.sync.dma_start(out=q_tiles[1][:, 0:512], in_=qT[1][:, 0:512])
            nc.sync.dma_start(out=q_tiles[0][:, 512:], in_=qT[0][:, 512:])
            nc.sync.dma_start(out=q_tiles[1][:, 512:], in_=qT[1][:, 512:])
            nc.sync.dma_start(out=mask_sb, in_=mask_dram)
            nc.sync.dma_start(out=kv_sb[0][0][:, 512:], in_=kT[0][:, 512:])
            nc.scalar.dma_start(
                out=kv_sb[0][1].rearrange("p (t c) -> p t c", t=NT)[:, 0:2, :],
                in_=v1[0].rearrange("t p c -> p t c")[:, 0:2, :],
            )
            nc.scalar.dma_start(
                out=kv_sb[0][1].rearrange("p (t c) -> p t c", t=NT)[:, 2:, :],
                in_=v1[0].rearrange("t p c -> p t c")[:, 2:, :],
            )
            load_q(2)
            load_q(3)
            load_kv(1)

            for h0 in range(0, H, 2):
                hs = (h0, h0 + 1)
                last = h0 == H - 2
                kvh = h0 // G
                kT_sb, v1_sb = kv_sb[kvh]
                load_q(h0 + 2)
                load_q(h0 + 3)
                if h0 % G == 0 and kvh + 2 < KVH:
                    load_kv(kvh + 2)
                groups = GROUPS_LAST if last else GROUPS
                p_loc = {h: {} for h in hs}
                d_sb = {h: {} for h in hs}
                osb_c = {
                    h: {
                        qc: opool.tile(
                            [128, 512], BF, tag="o", name=f"o_{h}_{qc}"
                        )
                        for qc in range(2)
                    }
                    for h in hs
                }
                osb_n = {h: {0: 0, 1: 0} for h in hs}

                po2 = {}

                def pv_run(h, qc, qt, start_kt=0, stop_kt=None):
                    # accumulate P.T @ [V|1] over qt's k tiles back-to-back;
                    # two q-tiles share one psum bank (single start=True per
                    # bank), reciprocal batched over both rowsums
                    if qt % 2 == 0 and start_kt == 0:
                        po2[(h, qt // 2)] = popool.tile(
                            [128, 258], F32, tag="po", name=f"po_{h}_{qt}"
                        )
                    po = po2[(h, qt // 2)]
                    base = (qt % 2) * 129
                    end_kt = qt + 1 if stop_kt is None else stop_kt
                    for kt in range(start_kt, end_kt):
                        if kt == qt:
                            lhsT = d_sb[h][(qc, kt)]
                        else:
                            t, pb = p_loc[h][(qc, kt)]
                            q_off = max(0, kt - qc * 4)
                            j = qt - qc * 4
                            lhsT = t[
                                :,
                                pb + (j - q_off) * 128 : pb
                                + (j - q_off) * 128
                                + 128,
                            ]
                        nc.tensor.matmul(
                            po[:, base : base + 129],
                            lhsT=lhsT,
                            rhs=v1_sb[:, kt * VW : kt * VW + 129],
                            start=(kt == 0 and qt % 2 == 0 and start_kt == 0),
                            stop=(kt == qt),
                            skip_group_check=True,
                        )
                    if stop_kt is not None and stop_kt <= qt:
                        return  # partial pre-accumulation; resumed later
                    if qt % 2 == 0:
                        return
                    recip = rpool.tile([128, 2], F32, tag="r", name=f"r_{h}_{qt}")
                    nc.vector.reciprocal(
                        recip, po.rearrange("p (a b) -> p a b", a=2)[:, :, 128]
                    )
                    # normalize BOTH q-tiles of the pair in one DVE op:
                    # po viewed [128, 2, 128] times recip broadcast along d
                    po3 = po.rearrange("p (a b) -> p a b", a=2)[:, :, 0:128]
                    rc3 = recip.rearrange("p (a b) -> p a b", b=1).broadcast_to(
                        [128, 2, 128]
                    )
                    j = qt - qc * 4
                    if last and qc == 1:
                        # tail: per-pair store alternating over the two
                        # now-idle DMA rings
                        osb = opool.tile(
                            [128, 256], BF, tag="o", name=f"ot_{h}_{qt}"
                        )
                        nc.vector.tensor_mul(
                            osb.rearrange("p (a b) -> p a b", a=2), po3, rc3
                        )
                        ring = nc.sync if (qt // 2 + h) % 2 == 0 else nc.gpsimd
                        ring.dma_start(
                            out=out[
                                h, (qt - 1) * 128 : (qt + 1) * 128, :
                            ].rearrange("(t p) d -> p t d", p=128),
                            in_=osb.rearrange("p (t d) -> p t d", t=2),
                        )
                        return
                    nc.vector.tensor_mul(
                        osb_c[h][qc][
                            :, (j - 1) * 128 : (j + 1) * 128
                        ].rearrange("p (a b) -> p a b", a=2),
                        po3,
                        rc3,
                    )
                    osb_n[h][qc] += 2
                    if osb_n[h][qc] == 4:
                        # one batched store per (head, chunk) from the GpSimd
                        # sequencer; keeps the Sync HWDGE ring free for loads
                        nc.gpsimd.dma_start(
                            out=out[h, qc * 512 : (qc + 1) * 512, :].rearrange(
                                "(t p) d -> p t d", p=128
                            ),
                            in_=osb_c[h][qc].rearrange("p (t d) -> p t d", t=4),
                        )

                pending = []
                for gi, (qc, kts, tw) in enumerate(groups):
                    # scores for both heads: one psum tile per (head, group)
                    ps_t = {}
                    for h in hs:
                        ps = pspool.tile(
                            [128, 1536], F32, tag="ps",
                            name=f"ps_{h}_{qc}_{kts[0][0]}",
                        )
                        ps_t[h] = ps
                        for kt, q_off, w, off in kts:
                            nc.tensor.matmul(
                                ps[:, off : off + w],
                                lhsT=kT_sb[:, kt * 128 : kt * 128 + 128],
                                rhs=q_tiles[h][
                                    :, qc * 512 + q_off * 128 : qc * 512 + 512
                                ],
                                start=True,
                                stop=True,
                                skip_group_check=True,
                            )
                    # last unit, final group: pre-accumulate qt6/qt7 over
                    # kt0..5 now so only the diagonal matmuls trail the
                    # final exp (shorter kernel tail); runs during the exps
                    if last and gi == 4:
                        for h in hs:
                            for qt in (6, 7):
                                pv_run(h, 1, qt, stop_kt=6)
                    # one wide exp per (head, group); ACT is the bottleneck,
                    # so the qc1 kt0-2 group of the pair's first head runs a
                    # bf16-Schraudolph exp on the (otherwise idle) DVE:
                    # bits = rne(s*128*log2e + (127*128 - 7.4)), bitcast bf16
                    # (rel err ~1.8% RMS; cancels in softmax num/denom)
                    for h in hs:
                        if gi == 2 and h == h0:
                            p_i16 = ppool.tile(
                                [128, tw], mybir.dt.int16, tag="p",
                                name=f"p_{h}_{qc}_{kts[0][0]}",
                            )
                            nc.vector.tensor_scalar(
                                p_i16,
                                ps_t[h][:, 0:tw],
                                184.6644353,
                                16248.6,
                                mybir.AluOpType.mult,
                                mybir.AluOpType.add,
                            )
                            p_sb = p_i16.bitcast(BF)
                        else:
                            p_sb = ppool.tile(
                                [128, tw], BF, tag="p",
                                name=f"p_{h}_{qc}_{kts[0][0]}",
                            )
                            # P = exp(scores); scale pre-folded into q on host
                            nc.scalar.activation(
                                p_sb, ps_t[h][:, 0:tw],
                                mybir.ActivationFunctionType.Exp,
                            )
                        for kt, q_off, w, off in kts:
                            p_loc[h][(qc, kt)] = (p_sb, off)
                            if kt >= qc * 4:  # diagonal: upper-tri mask
                                dt_ = dpool.tile(
                                    [128, 128], BF, tag="d",
                                    name=f"d_{h}_{qc}_{kt}",
                                )
                                nc.vector.tensor_mul(
                                    dt_, p_sb[:, off : off + 128], mask_sb
                                )
                                d_sb[h][(qc, kt)] = dt_
                    # emit PV runs one group late so the next group's QK +
                    # exp stay ahead of the PV burst on the PE stream
                    # (eager on the last unit to shorten the kernel tail)
                    for args in pending:
                        pv_run(*args)
                    pending = sorted(
                        (h, qc, kt)
                        for h in hs
                        for kt, q_off, w, off in kts
                        if kt >= qc * 4
                    )
                    if last:
                        for h3, qc3, qt3 in pending:
                            pv_run(h3, qc3, qt3, start_kt=6 if qt3 >= 6 else 0)
                        pending = []
                for args in pending:
                    pv_run(*args)

    nc.compile()
    return nc


def _get_nc():
    global _NC
    if _NC is None:
        _NC = _build_nc()
    return _NC


def make_in_maps(q, k, v, k_cache, v_cache, slot_mapping, block_tables):
    nb, bs, kvh, d = k_cache.shape
    # store_kvcache scatter (mirrors reference semantics on host)
    kc = k_cache.reshape(nb * bs, kvh, d).copy()
    vc = v_cache.reshape(nb * bs, kvh, d).copy()
    kc[slot_mapping] = k
    vc[slot_mapping] = v
    b, mb = block_tables.shape
    s = q.shape[0] // b
    pos = np.arange(s)
    slot_grid = block_tables[:, pos // bs] * bs + (pos % bs)  # [B, S]
    kf = kc[slot_grid]  # [B, S, KVH, D]
    vf = vc[slot_grid]
    qb = q.reshape(b, s, H, D)

    bf16 = ml_dtypes.bfloat16
    in_maps = []
    for i in range(b):
        qTi = np.ascontiguousarray(
            qb[i].transpose(1, 2, 0) * np.float32(SCALE)
        ).astype(bf16)
        kTi = np.ascontiguousarray(kf[i].transpose(1, 2, 0)).astype(bf16)
        vh = vf[i].transpose(1, 0, 2).reshape(KVH, NT, 128, D)
        v1i = np.zeros((KVH, NT, 128, VW), dtype=bf16)
        v1i[..., :D] = vh.astype(bf16)
        v1i[..., D] = 1.0
        in_maps.append({"qT": qTi, "kT": kTi, "v1": v1i})
    return in_maps


def kernel(q, k, v, k_cache, v_cache, slot_mapping, block_tables):
    # accept jax or numpy inputs
    q = np.asarray(q)
    k = np.asarray(k)
    v = np.asarray(v)
    k_cache = np.asarray(k_cache)
    v_cache = np.asarray(v_cache)
    slot_mapping = np.asarray(slot_mapping)
    block_tables = np.asarray(block_tables)
    out_dtype = q.dtype
    in_maps = make_in_maps(q, k, v, k_cache, v_cache, slot_mapping, block_tables)
    nc = _get_nc()
    res = run_bass_kernel_spmd(nc, in_maps, core_ids=list(range(8)))
    outs = [
        np.asarray(res.results[i]["out"]).transpose(1, 0, 2) for i in range(B)
    ]  # [S, H, D]
    return np.concatenate(outs, axis=0).astype(out_dtype)


# revision 13
# speedup vs baseline: 1.1862x; 1.1862x over previous
"""Paged-KV-cache causal GQA attention on 8 TRN2 NeuronCores.

Problem shape (hardcoded): B=8 seqs x S=1024 tokens, H=32 q-heads,
KVH=8 kv-heads (GQA group 4), D=128, block_size=256, 40 cache blocks.

Sharding: data parallel, one sequence per core. Host does the
store_kvcache scatter + block-table gather (layout work) and per-core
layout prep (head-major transposes + bf16 cast, scale folded into q);
each core runs causal flash attention for its sequence over all 32
heads.

Device algorithm per (head, q-chunk of 512), two heads interleaved:
  warmup: 8 dummy matmuls on a memset tile bridge the initial DMA
          wait so the PE HAM clock-gate reaches 8/8 before real work
  phase 1 (per k-tile group of 2-4 tiles): scores^T[k,q] = K^T.T @ Q^T
           (PE, bf16) packed into one [128, <=1536] psum tile (regions
           never cross a psum bank boundary); P = exp(scores) in ONE
           wide ACT inst per group (ACT is the global bottleneck:
           fewer/wider exps cut its ~176ns/inst overhead); one group
           per pair instead runs a bf16-Schraudolph exp on the DVE to
           offload ACT; diagonal tiles masked into separate tiles (DVE)
  phase 2 (per q-tile): O[q,0:128]+rowsum[q] = P.T @ [V|1] accumulated
           over its k tiles back-to-back (PE), then out = O * (1/rowsum)
           (one DVE op per q-tile pair via broadcast AP, bf16 out) and
           DMA out (bf16 halves store traffic; host upcasts to f32).
Score psum double-buffered 2x3 banks + po double-buffered 2x1 bank.
The head phase is HBM-bound (8 cores burst-load at once): critical
bytes go on one ring in strict need order; v1[0] rides the scalar ring.
"""

import sys

import numpy as np
import ml_dtypes

sys.path.insert(0, "/opt/trn_rl_repo")

import concourse.bass as bass  # noqa: E402
import concourse.mybir as mybir  # noqa: E402
import concourse.tile as tile  # noqa: E402
from concourse import bacc  # noqa: E402
from concourse.bass_utils import run_bass_kernel_spmd  # noqa: E402

B, S = 8, 1024
H, KVH, D = 32, 8, 128
G = H // KVH
NT = S // 128  # 8 k/q tiles of 128 per sequence
VW = 132  # v tile row: 128 v cols + ones col + pad
SCALE = 1.0 / float(np.sqrt(D))
BF = mybir.dt.bfloat16
F32 = mybir.dt.float32
_NC = None

# k-tile groups per q-chunk: (qc, [(kt, q_off, width, psum_off), ...], tw).
# psum_off values are arranged so no matmul output region crosses a 2KB
# (512-f32) psum bank boundary.
GROUPS = [
    (0, [(0, 0, 512, 0), (1, 1, 384, 512), (3, 3, 128, 896), (2, 2, 256, 1024)], 1280),
    (1, [(0, 0, 512, 0), (1, 0, 512, 512), (2, 0, 512, 1024)], 1536),
    (1, [(3, 0, 512, 0), (4, 0, 512, 512)], 1024),
    (1, [(5, 1, 384, 0), (7, 3, 128, 384), (6, 2, 256, 512)], 768),
]
# last unit: split kt5 out so qt5's PV fully finalizes before qt6/7's
# pre-accumulation allocates its psum slot (pool-rotation safety), and
# only the kt6/kt7 diagonal matmuls trail the final exp.
GROUPS_LAST = [
    GROUPS[0],
    GROUPS[1],
    GROUPS[2],
    (1, [(5, 1, 384, 0)], 384),
    (1, [(6, 2, 256, 0), (7, 3, 128, 256)], 384),
]


def _build_nc():
    nc = bacc.Bacc("TRN2", target_bir_lowering=False, debug=False, num_devices=8)
    qT = nc.dram_tensor("qT", [H, D, S], BF, kind="ExternalInput").ap()
    kT = nc.dram_tensor("kT", [KVH, D, S], BF, kind="ExternalInput").ap()
    v1 = nc.dram_tensor("v1", [KVH, NT, 128, VW], BF, kind="ExternalInput").ap()
    out = nc.dram_tensor("out", [H, S, D], BF, kind="ExternalOutput").ap()
    mask_np = np.triu(np.ones((128, 128), dtype=ml_dtypes.bfloat16))
    mask_dram = nc.inline_tensor(mask_np, "tri_mask").ap()

    with tile.TileContext(nc) as tc:
        with (
            tc.tile_pool(name="singles", bufs=1) as singles,
            tc.tile_pool(name="qpool", bufs=6) as qpool,
            tc.tile_pool(name="ppool", bufs=16) as ppool,
            tc.tile_pool(name="dpool", bufs=22) as dpool,
            tc.tile_pool(name="opool", bufs=8) as opool,
            tc.tile_pool(name="rpool", bufs=10) as rpool,
            tc.tile_pool(name="pspool", bufs=2, space="PSUM") as pspool,
            tc.tile_pool(name="popool", bufs=2, space="PSUM") as popool,
        ):
            # --- HAM warmup: dummy matmuls with no data deps keep the
            # PE busy through the initial DMA wait so the clock gate is
            # at 8/8 when the first real matmul issues ---
            warm_sb = singles.tile([128, 256], BF, name="warm_sb")
            nc.vector.memset(warm_sb, 0.0)
            dummy_ps = popool.tile([128, 258], F32, tag="po", name="dummy_ps")
            for i in range(8):
                nc.tensor.matmul(
                    dummy_ps[:, 0:256],
                    lhsT=warm_sb[:, 0:128],
                    rhs=warm_sb,
                    start=True,
                    stop=True,
                    skip_group_check=True,
                )

            mask_sb = singles.tile([128, 128], BF)
            kv_sb = []
            for kvh in range(KVH):
                k_t = singles.tile([128, S], BF, name=f"kT_sb{kvh}", tag=f"kT{kvh}")
                v_t = singles.tile(
                    [128, NT * VW], BF, name=f"v1_sb{kvh}", tag=f"v1{kvh}"
                )
                kv_sb.append((k_t, v_t))

            def load_kv(kvh):
                # kT on the sync HWDGE ring; v1 on the gpsimd SWDGE ring so
                # the two streams' kickoffs and transfers run in parallel
                nc.sync.dma_start(out=kv_sb[kvh][0], in_=kT[kvh])
                nc.gpsimd.dma_start(
                    out=kv_sb[kvh][1].rearrange("p (t c) -> p t c", t=NT),
                    in_=v1[kvh].rearrange("t p c -> p t c"),
                )

            q_tiles = {}

            def load_q(h):
                if h < H and h not in q_tiles:
                    q_tiles[h] = qpool.tile([128, S], BF, tag="q", name=f"q_sb{h}")
                    nc.sync.dma_start(out=q_tiles[h], in_=qT[h])

            # fast start: the head phase is HBM-bandwidth-bound (all 8 cores
            # burst-load at once), so the critical bytes go on ONE ring in
            # strict need order and nothing else competes with them; only
            # v1[0] (needed ~1us later, for the first PVs) rides the
            # otherwise-idle scalar HWDGE ring
            q_tiles[0] = qpool.tile([128, S], BF, tag="q", name="q_sb0")
            q_tiles[1] = qpool.tile([128, S], BF, tag="q", name="q_sb1")
            nc.sync.dma_start(out=mask_sb, in_=mask_dram)
            nc.sync.dma_start(out=kv_sb[0][0][:, 0:512], in_=kT[0][:, 0:512])
            nc.sync.dma_start(out=q_tiles[0][:, 0:512], in_=qT[0][:, 0:512])
            nc.sync.dma_start(out=q_tiles[1][:, 0:512], in_=qT[1][:, 0:512])
            nc.sync.dma_start(out=q_tiles[0][:, 512:], in_=qT[0][:, 512:])
            nc.sync.dma_start(out=q_tiles[1][:, 512:], in_=qT[1][:, 512:])
            nc.sync.dma_start(out=kv_sb[0][0][:, 512:], in_=kT[0][:, 512:])
            nc.scalar.dma_start(
                out=kv_sb[0][1].rearrange("p (t c) -> p t c", t=NT)[:, 0:2, :],
                in_=v1[0].rearrange("t p c -> p t c")[:, 0:2, :],
            )
            nc.scalar.dma_start(
                out=kv_sb[0][1].rearrange("p (t c) -> p t c", t=NT)[:, 2:, :],
                in_=v1[0].rearrange("t p c -> p t c")[:, 2:, :],
            )
            load_q(2)
            load_q(3)
            load_kv(1)

            for h0 in range(0, H, 2):
                hs = (h0, h0 + 1)
                last = h0 == H - 2
                kvh = h0 // G
                kT_sb, v1_sb = kv_sb[kvh]
                load_q(h0 + 2)
                load_q(h0 + 3)
                if h0 % G == 0 and kvh + 2 < KVH:
                    load_kv(kvh + 2)
                groups = GROUPS_LAST if last else GROUPS
                p_loc = {h: {} for h in hs}
                d_sb = {h: {} for h in hs}
                osb_c = {
                    h: {
                        qc: opool.tile(
                            [128, 512], BF, tag="o", name=f"o_{h}_{qc}"
                        )
                        for qc in range(2)
                    }
                    for h in hs
                }
                osb_n = {h: {0: 0, 1: 0} for h in hs}

                po2 = {}

                def pv_run(h, qc, qt, start_kt=0, stop_kt=None):
                    # accumulate P.T @ [V|1] over qt's k tiles back-to-back;
                    # two q-tiles share one psum bank (single start=True per
                    # bank), reciprocal batched over both rowsums
                    if qt % 2 == 0 and start_kt == 0:
                        po2[(h, qt // 2)] = popool.tile(
                            [128, 258], F32, tag="po", name=f"po_{h}_{qt}"
                        )
                    po = po2[(h, qt // 2)]
                    base = (qt % 2) * 129
                    end_kt = qt + 1 if stop_kt is None else stop_kt
                    for kt in range(start_kt, end_kt):
                        if kt == qt:
                            lhsT = d_sb[h][(qc, kt)]
                        else:
                            t, pb = p_loc[h][(qc, kt)]
                            q_off = max(0, kt - qc * 4)
                            j = qt - qc * 4
                            lhsT = t[
                                :,
                                pb + (j - q_off) * 128 : pb
                                + (j - q_off) * 128
                                + 128,
                            ]
                        nc.tensor.matmul(
                            po[:, base : base + 129],
                            lhsT=lhsT,
                            rhs=v1_sb[:, kt * VW : kt * VW + 129],
                            start=(kt == 0 and qt % 2 == 0 and start_kt == 0),
                            stop=(kt == qt),
                            skip_group_check=True,
                        )
                    if stop_kt is not None and stop_kt <= qt:
                        return  # partial pre-accumulation; resumed later
                    if qt % 2 == 0:
                        return
                    recip = rpool.tile([128, 2], F32, tag="r", name=f"r_{h}_{qt}")
                    nc.vector.reciprocal(
                        recip, po.rearrange("p (a b) -> p a b", a=2)[:, :, 128]
                    )
                    # normalize BOTH q-tiles of the pair in one DVE op:
                    # po viewed [128, 2, 128] times recip broadcast along d
                    po3 = po.rearrange("p (a b) -> p a b", a=2)[:, :, 0:128]
                    rc3 = recip.rearrange("p (a b) -> p a b", b=1).broadcast_to(
                        [128, 2, 128]
                    )
                    j = qt - qc * 4
                    if last and qc == 1:
                        # tail: per-pair store alternating over the two
                        # now-idle DMA rings
                        osb = opool.tile(
                            [128, 256], BF, tag="o", name=f"ot_{h}_{qt}"
                        )
                        nc.vector.tensor_mul(
                            osb.rearrange("p (a b) -> p a b", a=2), po3, rc3
                        )
                        ring = nc.sync if (qt // 2 + h) % 2 == 0 else nc.gpsimd
                        ring.dma_start(
                            out=out[
                                h, (qt - 1) * 128 : (qt + 1) * 128, :
                            ].rearrange("(t p) d -> p t d", p=128),
                            in_=osb.rearrange("p (t d) -> p t d", t=2),
                        )
                        return
                    nc.vector.tensor_mul(
                        osb_c[h][qc][
                            :, (j - 1) * 128 : (j + 1) * 128
                        ].rearrange("p (a b) -> p a b", a=2),
                        po3,
                        rc3,
                    )
                    osb_n[h][qc] += 2
                    if osb_n[h][qc] == 4:
                        # one batched store per (head, chunk) from the GpSimd
                        # sequencer; keeps the Sync HWDGE ring free for loads
                        nc.gpsimd.dma_start(
                            out=out[h, qc * 512 : (qc + 1) * 512, :].rearrange(
                                "(t p) d -> p t d", p=128
                            ),
                            in_=osb_c[h][qc].rearrange("p (t d) -> p t d", t=4),
                        )

                pending = []
                for gi, (qc, kts, tw) in enumerate(groups):
                    # scores for both heads: one psum tile per (head, group)
                    ps_t = {}
                    for h in hs:
                        ps = pspool.tile(
                            [128, 1536], F32, tag="ps",
                            name=f"ps_{h}_{qc}_{kts[0][0]}",
                        )
                        ps_t[h] = ps
                        for kt, q_off, w, off in kts:
                            nc.tensor.matmul(
                                ps[:, off : off + w],
                                lhsT=kT_sb[:, kt * 128 : kt * 128 + 128],
                                rhs=q_tiles[h][
                                    :, qc * 512 + q_off * 128 : qc * 512 + 512
                                ],
                                start=True,
                                stop=True,
                                skip_group_check=True,
                            )
                    # last unit, final group: pre-accumulate qt6/qt7 over
                    # kt0..5 now so only the diagonal matmuls trail the
                    # final exp (shorter kernel tail); runs during the exps
                    if last and gi == 4:
                        for h in hs:
                            for qt in (6, 7):
                                pv_run(h, 1, qt, stop_kt=6)
                    # one wide exp per (head, group); ACT is the bottleneck,
                    # so the qc1 kt0-2 group of the pair's first head runs a
                    # bf16-Schraudolph exp on the (otherwise idle) DVE:
                    # bits = rne(s*128*log2e + (127*128 - 7.4)), bitcast bf16
                    # (rel err ~1.8% RMS; cancels in softmax num/denom)
                    for h in hs:
                        if gi == 2 and h == h0:
                            p_i16 = ppool.tile(
                                [128, tw], mybir.dt.int16, tag="p",
                                name=f"p_{h}_{qc}_{kts[0][0]}",
                            )
                            nc.vector.tensor_scalar(
                                p_i16,
                                ps_t[h][:, 0:tw],
                                184.6644353,
                                16248.6,
                                mybir.AluOpType.mult,
                                mybir.AluOpType.add,
                            )
                            p_sb = p_i16.bitcast(BF)
                        else:
                            p_sb = ppool.tile(
                                [128, tw], BF, tag="p",
                                name=f"p_{h}_{qc}_{kts[0][0]}",
                            )
                            # P = exp(scores); scale pre-folded into q on host
                            nc.scalar.activation(
                                p_sb, ps_t[h][:, 0:tw],
                                mybir.ActivationFunctionType.Exp,
                            )
                        for kt, q_off, w, off in kts:
                            p_loc[h][(qc, kt)] = (p_sb, off)
                            if kt >= qc * 4:  # diagonal: upper-tri mask
                                dt_ = dpool.tile(
                                    [128, 128], BF, tag="d",
                                    name=f"d_{h}_{qc}_{kt}",
                                )
                                nc.vector.tensor_mul(
                                    dt_, p_sb[:, off : off + 128], mask_sb
                                )
                                d_sb[h][(qc, kt)] = dt_
                    # emit PV runs one group late so the next group's QK +
                    # exp stay ahead of the PV burst on the PE stream
                    # (eager on the last unit to shorten the kernel tail)
                    for args in pending:
                        pv_run(*args)
                    pending = sorted(
                        (h, qc, kt)
                        for h in hs
                        for kt, q_off, w, off in kts
                        if kt >= qc * 4
                    )
                    if last:
                        for h3, qc3, qt3 in pending:
                            pv_run(h3, qc3, qt3, start_kt=6 if qt3 >= 6 else 0)
                        pending = []
                for args in pending:
                    pv_run(*args)

    nc.compile()
    return nc


def _get_nc():
    global _NC
    if _NC is None:
        _NC = _build_nc()
    return _NC


def make_in_maps(q, k, v, k_cache, v_cache, slot_mapping, block_tables):
    nb, bs, kvh, d = k_cache.shape
    # store_kvcache scatter (mirrors reference semantics on host)
    kc = k_cache.reshape(nb * bs, kvh, d).copy()
    vc = v_cache.reshape(nb * bs, kvh, d).copy()
    kc[slot_mapping] = k
    vc[slot_mapping] = v
    b, mb = block_tables.shape
    s = q.shape[0] // b
    pos = np.arange(s)
    slot_grid = block_tables[:, pos // bs] * bs + (pos % bs)  # [B, S]
    kf = kc[slot_grid]  # [B, S, KVH, D]
    vf = vc[slot_grid]
    qb = q.reshape(b, s, H, D)

    bf16 = ml_dtypes.bfloat16
    in_maps = []
    for i in range(b):
        qTi = np.ascontiguousarray(
            qb[i].transpose(1, 2, 0) * np.float32(SCALE)
        ).astype(bf16)
        kTi = np.ascontiguousarray(kf[i].transpose(1, 2, 0)).astype(bf16)
        vh = vf[i].transpose(1, 0, 2).reshape(KVH, NT, 128, D)
        v1i = np.zeros((KVH, NT, 128, VW), dtype=bf16)
        v1i[..., :D] = vh.astype(bf16)
        v1i[..., D] = 1.0
        in_maps.append({"qT": qTi, "kT": kTi, "v1": v1i})
    return in_maps


def kernel(q, k, v, k_cache, v_cache, slot_mapping, block_tables):
    # accept jax or numpy inputs
    q = np.asarray(q)
    k = np.asarray(k)
    v = np.asarray(v)
    k_cache = np.asarray(k_cache)
    v_cache = np.asarray(v_cache)
    slot_mapping = np.asarray(slot_mapping)
    block_tables = np.asarray(block_tables)
    out_dtype = q.dtype
    in_maps = make_in_maps(q, k, v, k_cache, v_cache, slot_mapping, block_tables)
    nc = _get_nc()
    res = run_bass_kernel_spmd(nc, in_maps, core_ids=list(range(8)))
    outs = [
        np.asarray(res.results[i]["out"]).transpose(1, 0, 2) for i in range(B)
    ]  # [S, H, D]
    return np.concatenate(outs, axis=0).astype(out_dtype)


# revision 17
# speedup vs baseline: 1.1936x; 1.0062x over previous
"""Paged-KV-cache causal GQA attention on 8 TRN2 NeuronCores.

Problem shape (hardcoded): B=8 seqs x S=1024 tokens, H=32 q-heads,
KVH=8 kv-heads (GQA group 4), D=128, block_size=256, 40 cache blocks.

Sharding: data parallel, one sequence per core. Host does the
store_kvcache scatter + block-table gather (layout work) and per-core
layout prep (head-major transposes + bf16 cast, scale folded into q);
each core runs causal flash attention for its sequence over all 32
heads.

Device algorithm per (head, q-chunk of 512), two heads interleaved:
  warmup: 8 dummy matmuls on a memset tile bridge the initial DMA
          wait so the PE HAM clock-gate reaches 8/8 before real work
  phase 1 (per k-tile group of 2-4 tiles): scores^T[k,q] = K^T.T @ Q^T
           (PE, bf16) packed into one [128, <=1536] psum tile (regions
           never cross a psum bank boundary); P = exp(scores) in ONE
           wide ACT inst per group (ACT is the global bottleneck:
           fewer/wider exps cut its ~176ns/inst overhead); one group
           per pair instead runs a bf16-Schraudolph exp on the DVE to
           offload ACT; diagonal tiles masked into separate tiles (DVE)
  phase 2 (per q-tile): O[q,0:128]+rowsum[q] = P.T @ [V|1] accumulated
           over its k tiles back-to-back (PE), then out = O * (1/rowsum)
           (one DVE op per q-tile pair via broadcast AP, bf16 out) and
           DMA out (bf16 halves store traffic; host upcasts to f32).
Score psum double-buffered 2x3 banks + po double-buffered 2x1 bank.
The head phase is HBM-bound (8 cores burst-load at once): critical
bytes go on one ring in strict need order; v1[0] rides the scalar ring.
"""

import sys

import numpy as np
import ml_dtypes

sys.path.insert(0, "/opt/trn_rl_repo")

import concourse.bass as bass  # noqa: E402
import concourse.mybir as mybir  # noqa: E402
import concourse.tile as tile  # noqa: E402
from concourse import bacc  # noqa: E402
from concourse.bass_utils import run_bass_kernel_spmd  # noqa: E402

B, S = 8, 1024
H, KVH, D = 32, 8, 128
G = H // KVH
NT = S // 128  # 8 k/q tiles of 128 per sequence
VW = 132  # v tile row: 128 v cols + ones col + pad
SCALE = 1.0 / float(np.sqrt(D))
BF = mybir.dt.bfloat16
F32 = mybir.dt.float32
_NC = None

# k-tile groups per q-chunk: (qc, [(kt, q_off, width, psum_off), ...], tw).
# psum_off values are arranged so no matmul output region crosses a 2KB
# (512-f32) psum bank boundary.
GROUPS = [
    (0, [(0, 0, 512, 0), (1, 1, 384, 512), (3, 3, 128, 896), (2, 2, 256, 1024)], 1280),
    (1, [(0, 0, 512, 0), (1, 0, 512, 512), (2, 0, 512, 1024)], 1536),
    (1, [(3, 0, 512, 0), (4, 0, 512, 512)], 1024),
    (1, [(5, 1, 384, 0), (7, 3, 128, 384), (6, 2, 256, 512)], 768),
]
# last unit: kt5/kt6/kt7 split into single-tile groups so each q-tile's
# PV finalizes as early as possible (pool-rotation safety) and only
# qt7's single diagonal matmul + finalize trail the very last exp.
GROUPS_LAST = [
    GROUPS[0],
    GROUPS[1],
    GROUPS[2],
    (1, [(5, 1, 384, 0)], 384),
    (1, [(6, 2, 256, 0)], 256),
    (1, [(7, 3, 128, 0)], 128),
]


def _build_nc():
    nc = bacc.Bacc("TRN2", target_bir_lowering=False, debug=False, num_devices=8)
    qT = nc.dram_tensor("qT", [H, D, S], BF, kind="ExternalInput").ap()
    kT = nc.dram_tensor("kT", [KVH, D, S], BF, kind="ExternalInput").ap()
    v1 = nc.dram_tensor("v1", [KVH, NT, 128, VW], BF, kind="ExternalInput").ap()
    out = nc.dram_tensor("out", [H, S, D], BF, kind="ExternalOutput").ap()
    mask_np = np.triu(np.ones((128, 128), dtype=ml_dtypes.bfloat16))
    mask_dram = nc.inline_tensor(mask_np, "tri_mask").ap()

    with tile.TileContext(nc) as tc:
        with (
            tc.tile_pool(name="singles", bufs=1) as singles,
            tc.tile_pool(name="qpool", bufs=6) as qpool,
            tc.tile_pool(name="ppool", bufs=16) as ppool,
            tc.tile_pool(name="dpool", bufs=22) as dpool,
            tc.tile_pool(name="opool", bufs=8) as opool,
            tc.tile_pool(name="rpool", bufs=10) as rpool,
            tc.tile_pool(name="pspool", bufs=2, space="PSUM") as pspool,
            tc.tile_pool(name="popool", bufs=2, space="PSUM") as popool,
        ):
            # --- HAM warmup: dummy matmuls with no data deps keep the
            # PE busy through the initial DMA wait so the clock gate is
            # at 8/8 when the first real matmul issues ---
            warm_sb = singles.tile([128, 256], BF, name="warm_sb")
            nc.vector.memset(warm_sb, 0.0)
            dummy_ps = popool.tile([128, 258], F32, tag="po", name="dummy_ps")
            for i in range(20):
                nc.tensor.matmul(
                    dummy_ps[:, 0:256],
                    lhsT=warm_sb[:, 0:128],
                    rhs=warm_sb,
                    start=True,
                    stop=True,
                    skip_group_check=True,
                )

            mask_sb = singles.tile([128, 128], BF)
            kv_sb = []
            for kvh in range(KVH):
                k_t = singles.tile([128, S], BF, name=f"kT_sb{kvh}", tag=f"kT{kvh}")
                v_t = singles.tile(
                    [128, NT * VW], BF, name=f"v1_sb{kvh}", tag=f"v1{kvh}"
                )
                kv_sb.append((k_t, v_t))

            def load_kv(kvh):
                # kT on the sync HWDGE ring; v1 on the gpsimd SWDGE ring so
                # the two streams' kickoffs and transfers run in parallel
                nc.sync.dma_start(out=kv_sb[kvh][0], in_=kT[kvh])
                nc.gpsimd.dma_start(
                    out=kv_sb[kvh][1].rearrange("p (t c) -> p t c", t=NT),
                    in_=v1[kvh].rearrange("t p c -> p t c"),
                )

            q_tiles = {}

            def load_q(h):
                if h < H and h not in q_tiles:
                    q_tiles[h] = qpool.tile([128, S], BF, tag="q", name=f"q_sb{h}")
                    nc.sync.dma_start(out=q_tiles[h], in_=qT[h])

            # fast start: the head phase is HBM-bandwidth-bound (all 8 cores
            # burst-load at once), so the critical bytes go on ONE ring in
            # strict need order and nothing else competes with them; only
            # v1[0] (needed ~1us later, for the first PVs) rides the
            # otherwise-idle scalar HWDGE ring
            q_tiles[0] = qpool.tile([128, S], BF, tag="q", name="q_sb0")
            q_tiles[1] = qpool.tile([128, S], BF, tag="q", name="q_sb1")
            nc.sync.dma_start(out=mask_sb, in_=mask_dram)
            nc.sync.dma_start(out=kv_sb[0][0][:, 0:512], in_=kT[0][:, 0:512])
            nc.sync.dma_start(out=q_tiles[0][:, 0:512], in_=qT[0][:, 0:512])
            nc.sync.dma_start(out=q_tiles[1][:, 0:512], in_=qT[1][:, 0:512])
            nc.sync.dma_start(out=q_tiles[0][:, 512:], in_=qT[0][:, 512:])
            nc.sync.dma_start(out=q_tiles[1][:, 512:], in_=qT[1][:, 512:])
            nc.sync.dma_start(out=kv_sb[0][0][:, 512:], in_=kT[0][:, 512:])
            nc.scalar.dma_start(
                out=kv_sb[0][1].rearrange("p (t c) -> p t c", t=NT)[:, 0:2, :],
                in_=v1[0].rearrange("t p c -> p t c")[:, 0:2, :],
            )
            nc.scalar.dma_start(
                out=kv_sb[0][1].rearrange("p (t c) -> p t c", t=NT)[:, 2:, :],
                in_=v1[0].rearrange("t p c -> p t c")[:, 2:, :],
            )
            load_q(2)
            load_q(3)
            load_kv(1)

            for h0 in range(0, H, 2):
                hs = (h0, h0 + 1)
                last = h0 == H - 2
                kvh = h0 // G
                kT_sb, v1_sb = kv_sb[kvh]
                load_q(h0 + 2)
                load_q(h0 + 3)
                if h0 % G == 0 and kvh + 2 < KVH:
                    load_kv(kvh + 2)
                groups = GROUPS_LAST if last else GROUPS
                p_loc = {h: {} for h in hs}
                d_sb = {h: {} for h in hs}
                osb_c = {
                    h: {
                        qc: opool.tile(
                            [128, 512], BF, tag="o", name=f"o_{h}_{qc}"
                        )
                        for qc in range(2)
                    }
                    for h in hs
                }
                osb_n = {h: {0: 0, 1: 0} for h in hs}

                po2 = {}

                def pv_run(h, qc, qt, start_kt=0, stop_kt=None):
                    # accumulate P.T @ [V|1] over qt's k tiles back-to-back;
                    # two q-tiles share one psum bank (single start=True per
                    # bank), reciprocal batched over both rowsums
                    if qt % 2 == 0 and start_kt == 0:
                        po2[(h, qt // 2)] = popool.tile(
                            [128, 258], F32, tag="po", name=f"po_{h}_{qt}"
                        )
                    po = po2[(h, qt // 2)]
                    base = (qt % 2) * 129
                    end_kt = qt + 1 if stop_kt is None else stop_kt
                    for kt in range(start_kt, end_kt):
                        if kt == qt:
                            lhsT = d_sb[h][(qc, kt)]
                        else:
                            t, pb = p_loc[h][(qc, kt)]
                            q_off = max(0, kt - qc * 4)
                            j = qt - qc * 4
                            lhsT = t[
                                :,
                                pb + (j - q_off) * 128 : pb
                                + (j - q_off) * 128
                                + 128,
                            ]
                        nc.tensor.matmul(
                            po[:, base : base + 129],
                            lhsT=lhsT,
                            rhs=v1_sb[:, kt * VW : kt * VW + 129],
                            start=(kt == 0 and qt % 2 == 0 and start_kt == 0),
                            stop=(kt == qt),
                            skip_group_check=True,
                        )
                    if stop_kt is not None and stop_kt <= qt:
                        return  # partial pre-accumulation; resumed later
                    if qt % 2 == 0:
                        return
                    recip = rpool.tile([128, 2], F32, tag="r", name=f"r_{h}_{qt}")
                    nc.vector.reciprocal(
                        recip, po.rearrange("p (a b) -> p a b", a=2)[:, :, 128]
                    )
                    # normalize BOTH q-tiles of the pair in one DVE op:
                    # po viewed [128, 2, 128] times recip broadcast along d
                    po3 = po.rearrange("p (a b) -> p a b", a=2)[:, :, 0:128]
                    rc3 = recip.rearrange("p (a b) -> p a b", b=1).broadcast_to(
                        [128, 2, 128]
                    )
                    j = qt - qc * 4
                    if last and qc == 1:
                        # tail: per-pair store alternating over the two
                        # now-idle DMA rings
                        osb = opool.tile(
                            [128, 256], BF, tag="o", name=f"ot_{h}_{qt}"
                        )
                        nc.vector.tensor_mul(
                            osb.rearrange("p (a b) -> p a b", a=2), po3, rc3
                        )
                        ring = nc.sync if (qt // 2 + h) % 2 == 0 else nc.gpsimd
                        ring.dma_start(
                            out=out[
                                h, (qt - 1) * 128 : (qt + 1) * 128, :
                            ].rearrange("(t p) d -> p t d", p=128),
                            in_=osb.rearrange("p (t d) -> p t d", t=2),
                        )
                        return
                    nc.vector.tensor_mul(
                        osb_c[h][qc][
                            :, (j - 1) * 128 : (j + 1) * 128
                        ].rearrange("p (a b) -> p a b", a=2),
                        po3,
                        rc3,
                    )
                    osb_n[h][qc] += 2
                    if osb_n[h][qc] == 4:
                        # one batched store per (head, chunk) from the GpSimd
                        # sequencer; keeps the Sync HWDGE ring free for loads
                        nc.gpsimd.dma_start(
                            out=out[h, qc * 512 : (qc + 1) * 512, :].rearrange(
                                "(t p) d -> p t d", p=128
                            ),
                            in_=osb_c[h][qc].rearrange("p (t d) -> p t d", t=4),
                        )

                pending = []
                for gi, (qc, kts, tw) in enumerate(groups):
                    # scores for both heads: one psum tile per (head, group)
                    ps_t = {}
                    for h in hs:
                        ps = pspool.tile(
                            [128, 1536], F32, tag="ps",
                            name=f"ps_{h}_{qc}_{kts[0][0]}",
                        )
                        ps_t[h] = ps
                        for kt, q_off, w, off in kts:
                            nc.tensor.matmul(
                                ps[:, off : off + w],
                                lhsT=kT_sb[:, kt * 128 : kt * 128 + 128],
                                rhs=q_tiles[h][
                                    :, qc * 512 + q_off * 128 : qc * 512 + 512
                                ],
                                start=True,
                                stop=True,
                                skip_group_check=True,
                            )
                    # last unit, final group: pre-accumulate qt7 over
                    # kt0..6 now so only its diagonal matmul trails the
                    # final exp (shorter kernel tail); runs during the exps
                    if last and gi == 5:
                        for h in hs:
                            pv_run(h, 1, 7, stop_kt=7)
                    # one wide exp per (head, group); ACT is the bottleneck,
                    # so the qc1 kt0-2 group of the pair's first head runs a
                    # bf16-Schraudolph exp on the (otherwise idle) DVE:
                    # bits = rne(s*128*log2e + (127*128 - 7.4)), bitcast bf16
                    # (rel err ~1.8% RMS; cancels in softmax num/denom)
                    for h in hs:
                        if gi == 2 and h == h0:
                            p_i16 = ppool.tile(
                                [128, tw], mybir.dt.int16, tag="p",
                                name=f"p_{h}_{qc}_{kts[0][0]}",
                            )
                            nc.vector.tensor_scalar(
                                p_i16,
                                ps_t[h][:, 0:tw],
                                184.6644353,
                                16248.6,
                                mybir.AluOpType.mult,
                                mybir.AluOpType.add,
                            )
                            p_sb = p_i16.bitcast(BF)
                        else:
                            p_sb = ppool.tile(
                                [128, tw], BF, tag="p",
                                name=f"p_{h}_{qc}_{kts[0][0]}",
                            )
                            # P = exp(scores); scale pre-folded into q on host
                            nc.scalar.activation(
                                p_sb, ps_t[h][:, 0:tw],
                                mybir.ActivationFunctionType.Exp,
                            )
                        for kt, q_off, w, off in kts:
                            p_loc[h][(qc, kt)] = (p_sb, off)
                            if kt >= qc * 4:  # diagonal: upper-tri mask
                                dt_ = dpool.tile(
                                    [128, 128], BF, tag="d",
                                    name=f"d_{h}_{qc}_{kt}",
                                )
                                nc.vector.tensor_mul(
                                    dt_, p_sb[:, off : off + 128], mask_sb
                                )
                                d_sb[h][(qc, kt)] = dt_
                    # emit PV runs one group late so the next group's QK +
                    # exp stay ahead of the PV burst on the PE stream
                    # (eager on the last unit to shorten the kernel tail)
                    for args in pending:
                        pv_run(*args)
                    pending = sorted(
                        (h, qc, kt)
                        for h in hs
                        for kt, q_off, w, off in kts
                        if kt >= qc * 4
                    )
                    if last and gi >= 3:
                        for h3, qc3, qt3 in pending:
                            pv_run(h3, qc3, qt3, start_kt=7 if qt3 == 7 else 0)
                        pending = []
                for args in pending:
                    pv_run(*args)

    nc.compile()
    return nc


def _get_nc():
    global _NC
    if _NC is None:
        _NC = _build_nc()
    return _NC


def make_in_maps(q, k, v, k_cache, v_cache, slot_mapping, block_tables):
    nb, bs, kvh, d = k_cache.shape
    # store_kvcache scatter (mirrors reference semantics on host)
    kc = k_cache.reshape(nb * bs, kvh, d).copy()
    vc = v_cache.reshape(nb * bs, kvh, d).copy()
    kc[slot_mapping] = k
    vc[slot_mapping] = v
    b, mb = block_tables.shape
    s = q.shape[0] // b
    pos = np.arange(s)
    slot_grid = block_tables[:, pos // bs] * bs + (pos % bs)  # [B, S]
    kf = kc[slot_grid]  # [B, S, KVH, D]
    vf = vc[slot_grid]
    qb = q.reshape(b, s, H, D)

    bf16 = ml_dtypes.bfloat16
    in_maps = []
    for i in range(b):
        qTi = np.ascontiguousarray(
            qb[i].transpose(1, 2, 0) * np.float32(SCALE)
        ).astype(bf16)
        kTi = np.ascontiguousarray(kf[i].transpose(1, 2, 0)).astype(bf16)
        vh = vf[i].transpose(1, 0, 2).reshape(KVH, NT, 128, D)
        v1i = np.zeros((KVH, NT, 128, VW), dtype=bf16)
        v1i[..., :D] = vh.astype(bf16)
        v1i[..., D] = 1.0
        in_maps.append({"qT": qTi, "kT": kTi, "v1": v1i})
    return in_maps


def kernel(q, k, v, k_cache, v_cache, slot_mapping, block_tables):
    # accept jax or numpy inputs
    q = np.asarray(q)
    k = np.asarray(k)
    v = np.asarray(v)
    k_cache = np.asarray(k_cache)
    v_cache = np.asarray(v_cache)
    slot_mapping = np.asarray(slot_mapping)
    block_tables = np.asarray(block_tables)
    out_dtype = q.dtype
    in_maps = make_in_maps(q, k, v, k_cache, v_cache, slot_mapping, block_tables)
    nc = _get_nc()
    res = run_bass_kernel_spmd(nc, in_maps, core_ids=list(range(8)))
    outs = [
        np.asarray(res.results[i]["out"]).transpose(1, 0, 2) for i in range(B)
    ]  # [S, H, D]
    return np.concatenate(outs, axis=0).astype(out_dtype)
